# revision 1
# baseline (speedup 1.0000x reference)
"""DeepFM Trainium2 kernel — 8-core SPMD, batch-sharded.

Strategy: shard the batch (16384 -> 8 x 2048); replicate the (host-packed,
bf16) embedding table and MLP weights on every core.  Per core:
  - indirect-DMA gather of 65-bf16-element rows ([64 emb | 1 lin]) from a
    combined [F*V, 96]-strided table, sample-major in SBUF
  - PE transposes to feature-major h^T [1728, batch]
  - bf16 MLP (1728->1024->512->256->1) feature-major, fp32 PSUM accumulate
  - FM second order via stacked-identity matmuls: A=sum_f e, B=sum_f e^2,
    second = 0.5*sum_d(A^2 - B) folded into the final 1-row matmul
  - first order: fp32 matmul of [Wd|Wld] against dense^T; lin-sum and
    bias/bld folded into the final matmul's contraction rows
"""

import numpy as np
import ml_dtypes

B, F, V, D, ND = 16384, 26, 100000, 64, 13
H1, H2, H3 = 1024, 512, 256
NCORES = 8
BS = B // NCORES            # 2048 samples per core
SUB = 128                   # gather sub-tile (samples)
NT = 512                    # outer batch tile (matmul N)
NOUT = BS // NT             # 4 outer tiles per core
NSUB = NT // SUB            # 4 sub-tiles per outer tile
ROWE = D + 1                # gathered elements per row (64 emb + 1 lin)
RSTRIDE = 96                # table row stride in elements (192B, 64B-aligned)
FV = F * V
HTOT = (F + 1) * D          # 1728
NCH = 14                    # ceil(1728/128); chunk 13 has 64 rows
KFIN = 97                   # final misc matmul contraction: 64 C + 26 lin + pad + bias@96

_cache = {}


def _build_nc(reps=1):
    import concourse.bass as bass
    import concourse.bacc as bacc
    import concourse.mybir as mybir
    import concourse.tile as tile

    dt = mybir.dt
    nc = bacc.Bacc()

    denseT = nc.declare_dram_parameter("denseT", [ND, BS], dt.float32, isOutput=False)
    idx = nc.declare_dram_parameter("idx", [BS, F], dt.int32, isOutput=False)
    table = nc.declare_dram_parameter("table", [FV, RSTRIDE], dt.bfloat16, isOutput=False)
    wdcat = nc.declare_dram_parameter("wdcat", [ND, D], dt.float32, isOutput=False)
    wld = nc.declare_dram_parameter("wld", [ND, 1], dt.float32, isOutput=False)
    w1 = nc.declare_dram_parameter("w1", [HTOT, H1], dt.bfloat16, isOutput=False)
    w2 = nc.declare_dram_parameter("w2", [H1, H2], dt.bfloat16, isOutput=False)
    w3 = nc.declare_dram_parameter("w3", [H2, H3], dt.bfloat16, isOutput=False)
    wout = nc.declare_dram_parameter("wout", [H3, 1], dt.bfloat16, isOutput=False)
    coeff = nc.declare_dram_parameter("coeff", [128, 1], dt.float32, isOutput=False)
    stacki = nc.declare_dram_parameter("stacki", [128, D], dt.bfloat16, isOutput=False)
    ident = nc.declare_dram_parameter("ident", [128, 128], dt.bfloat16, isOutput=False)
    out = nc.declare_dram_parameter("out", [NOUT, NT], dt.float32, isOutput=True)

    with tile.TileContext(nc) as tc:
        with (
            tc.tile_pool(name="const", bufs=1) as constp,
            tc.tile_pool(name="g", bufs=8) as gp,
            tc.tile_pool(name="ht", bufs=2) as htp,
            tc.tile_pool(name="act", bufs=2) as actp,
            tc.tile_pool(name="sq", bufs=3) as sqp,
            tc.tile_pool(name="misc", bufs=2) as miscp,
            tc.tile_pool(name="pdcat", bufs=1, space="PSUM") as pdcatp,
            tc.tile_pool(name="ptr", bufs=2, space="PSUM") as ptrp,
            tc.tile_pool(name="pab", bufs=1, space="PSUM") as pabp,
            tc.tile_pool(name="pfin", bufs=1, space="PSUM") as pfinp,
            tc.tile_pool(name="pl", bufs=2, space="PSUM") as plp,
        ):
            # ---- constants / weights, loaded once ----
            densesb = constp.tile([ND, BS], dt.float32)
            nc.sync.dma_start(out=densesb[:], in_=denseT[:])
            idxsb = constp.tile([SUB, (BS // SUB) * F], dt.int32)
            nc.sync.dma_start(
                out=idxsb[:].rearrange("p (st f) -> p st f", f=F),
                in_=idx[:].rearrange("(st p) f -> p st f", p=SUB),
            )
            wdcatsb = constp.tile([ND, D], dt.float32)
            nc.sync.dma_start(out=wdcatsb[:], in_=wdcat[:])
            wldsb = constp.tile([ND, 1], dt.float32)
            nc.sync.dma_start(out=wldsb[:], in_=wld[:])
            w1sb = constp.tile([128, NCH * H1], dt.bfloat16)
            for c in range(NCH):
                kc = min(128, HTOT - c * 128)
                nc.sync.dma_start(
                    out=w1sb[:kc, c * H1:(c + 1) * H1],
                    in_=w1[c * 128:c * 128 + kc, :],
                )
            w2sb = constp.tile([128, (H1 // 128) * H2], dt.bfloat16)
            for c in range(H1 // 128):
                nc.sync.dma_start(
                    out=w2sb[:, c * H2:(c + 1) * H2],
                    in_=w2[c * 128:(c + 1) * 128, :],
                )
            w3sb = constp.tile([128, (H2 // 128) * H3], dt.bfloat16)
            for c in range(H2 // 128):
                nc.sync.dma_start(
                    out=w3sb[:, c * H3:(c + 1) * H3],
                    in_=w3[c * 128:(c + 1) * 128, :],
                )
            woutsb = constp.tile([128, H3 // 128], dt.bfloat16)
            nc.sync.dma_start(
                out=woutsb[:], in_=wout[:].rearrange("(c p) one -> p (c one)", p=128)
            )
            coeffsb = constp.tile([128, 1], dt.float32)
            nc.sync.dma_start(out=coeffsb[:], in_=coeff[:])
            stackisb = constp.tile([128, D], dt.bfloat16)
            nc.sync.dma_start(out=stackisb[:], in_=stacki[:])
            identsb = constp.tile([128, 128], dt.bfloat16)
            nc.sync.dma_start(out=identsb[:], in_=ident[:])

            # ================= software-pipelined tile loop =================
            # Stage k runs the "front" of tile cur (gathers, dense matmul,
            # PE transposes to feature-major) interleaved with the "compute"
            # of tile prev (FM sums + MLP + final row), so the PE never has
            # long idle gaps (keeps HAM at 8/8) and DVE copies hide under
            # matmul phases.
            tiles = [t for _ in range(reps) for t in range(NOUT)]
            steps = [(tiles[k], tiles[k - 1] if k else None) for k in range(len(tiles))]
            steps.append((None, tiles[-1]))
            H = {}   # live handles for the in-flight tile

            def chunk_feats(c):
                if c == 0:
                    return D, D, [(D, 0)]
                if c == NCH - 1:
                    return 0, D, [(0, F - 1)]
                return 0, 128, [(0, 2 * c - 1), (D, 2 * c)]

            for cur, prev in steps:
                P = H.get(prev)
                if cur is not None:
                    # emb rows gathered feature-contiguous (64 elems) so a
                    # feature PAIR is one contiguous [128, 128] block; lin
                    # values gathered separately (element_offset=64)
                    gtiles = [gp.tile([SUB, F * D], dt.bfloat16, tag="g",
                                      name=f"g{s}")
                              for s in range(NSUB)]
                    gl = gp.tile([SUB, NSUB * F], dt.bfloat16, tag="gl")
                    for f in range(F):
                        for s in range(NSUB):
                            st = cur * NSUB + s
                            nc.gpsimd.indirect_dma_start(
                                out=gtiles[s][:, f * D:(f + 1) * D],
                                out_offset=None,
                                in_=table[:],
                                in_offset=bass.IndirectOffsetOnAxis(
                                    ap=idxsb[:, st * F + f:st * F + f + 1], axis=0
                                ),
                            )
                    for f in range(F):
                        for s in range(NSUB):
                            st = cur * NSUB + s
                            nc.gpsimd.indirect_dma_start(
                                out=gl[:, s * F + f:s * F + f + 1],
                                out_offset=None,
                                in_=table[:],
                                in_offset=bass.IndirectOffsetOnAxis(
                                    ap=idxsb[:, st * F + f:st * F + f + 1], axis=0
                                ),
                                element_offset=D,
                            )
                    pdcat = pdcatp.tile([D, NT], dt.float32)
                    nc.tensor.matmul(
                        out=pdcat[:],
                        lhsT=wdcatsb[:],
                        rhs=densesb[:, cur * NT:(cur + 1) * NT],
                        start=True,
                        stop=True,
                    )
                    ht = htp.tile([128, NCH * NT], dt.bfloat16, tag="ht")
                    nc.scalar.activation(
                        out=ht[0:D, 0:NT],
                        in_=pdcat[0:D, :],
                        func=mybir.ActivationFunctionType.Copy,
                    )
                    cext = miscp.tile([128, NT], dt.float32, tag="cext")
                    nc.vector.memset(cext[D:128, :], 0.0)
                    nc.vector.memset(cext[96:97, :], 1.0)
                    C = {"g": gtiles, "gl": gl, "ht": ht, "cext": cext}
                    H[cur] = C
                if P is not None:
                    pa = pabp.tile([D, NT], dt.float32, tag="pa")
                    pb = pabp.tile([D, NT], dt.float32, tag="pb")
                    h1t = actp.tile([128, (H1 // 128) * NT], dt.bfloat16, tag="h1t")

                # interleaved: per chunk c, prev's sq/A/B + cur's transposes,
                # plus one L1 m-tile for the first 8 chunks
                for c in range(NCH):
                    kc = min(128, HTOT - c * 128)
                    if P is not None:
                        htc = P["ht"][0:kc, c * NT:(c + 1) * NT]
                        sq = sqp.tile([128, NT], dt.bfloat16, tag="sq")
                        nc.vector.tensor_tensor(
                            out=sq[0:kc, :], in0=htc, in1=htc,
                            op=mybir.AluOpType.mult,
                        )
                        nc.tensor.matmul(
                            out=pa[:], lhsT=stackisb[0:kc, :], rhs=htc,
                            start=(c == 0), stop=(c == NCH - 1),
                        )
                        nc.tensor.matmul(
                            out=pb[:], lhsT=stackisb[0:kc, :], rhs=sq[0:kc, :],
                            start=(c == 0), stop=(c == NCH - 1),
                        )
                    if cur is not None:
                        # transpose via REGULAR matmul G.T @ I — unlike
                        # transpose-mode this counts as PE-busy for HAM
                        plo, kcc, feats = chunk_feats(c)
                        f0 = feats[0][1]
                        ptr = ptrp.tile([128, NT], dt.float32, tag="ptr")
                        for s in range(NSUB):
                            nc.tensor.matmul(
                                out=ptr[plo:plo + kcc, s * SUB:(s + 1) * SUB],
                                lhsT=C["g"][s][:, f0 * D:f0 * D + kcc],
                                rhs=identsb[:],
                                start=True,
                                stop=True,
                            )
                        nc.vector.tensor_copy(
                            out=C["ht"][plo:plo + kcc, c * NT:(c + 1) * NT],
                            in_=ptr[plo:plo + kcc, :],
                        )
                    if P is not None and c < H1 // 128:
                        m = c
                        pl = plp.tile([128, NT], dt.float32, tag="pl")
                        for cc in range(NCH):
                            kcc2 = min(128, HTOT - cc * 128)
                            nc.tensor.matmul(
                                out=pl[:],
                                lhsT=w1sb[0:kcc2, cc * H1 + m * 128:cc * H1 + (m + 1) * 128],
                                rhs=P["ht"][0:kcc2, cc * NT:(cc + 1) * NT],
                                start=(cc == 0),
                                stop=(cc == NCH - 1),
                            )
                        nc.scalar.activation(
                            out=h1t[:, m * NT:(m + 1) * NT],
                            in_=pl[:],
                            func=mybir.ActivationFunctionType.Relu,
                        )

                # cur: lin-row transposes (all gathers are done by now)
                if cur is not None:
                    pltr = ptrp.tile([128, NT], dt.float32, tag="ptr")
                    for s in range(NSUB):
                        nc.tensor.matmul(
                            out=pltr[D:D + F, s * SUB:(s + 1) * SUB],
                            lhsT=C["gl"][:, s * F:(s + 1) * F],
                            rhs=identsb[:],
                            start=True,
                            stop=True,
                        )
                    nc.vector.tensor_copy(
                        out=C["cext"][D:D + F, :], in_=pltr[D:D + F, :]
                    )

                if P is None:
                    continue

                # ---- prev: layers 2/3 ----
                h2t = actp.tile([128, (H2 // 128) * NT], dt.bfloat16, tag="h2t")
                for m in range(H2 // 128):
                    pl = plp.tile([128, NT], dt.float32, tag="pl")
                    for c in range(H1 // 128):
                        nc.tensor.matmul(
                            out=pl[:],
                            lhsT=w2sb[:, c * H2 + m * 128:c * H2 + (m + 1) * 128],
                            rhs=h1t[:, c * NT:(c + 1) * NT],
                            start=(c == 0),
                            stop=(c == H1 // 128 - 1),
                        )
                    nc.scalar.activation(
                        out=h2t[:, m * NT:(m + 1) * NT],
                        in_=pl[:],
                        func=mybir.ActivationFunctionType.Relu,
                    )
                # FM second-order combine, overlaps L3 on PE
                asq = miscp.tile([D, NT], dt.float32, tag="asq")
                nc.scalar.activation(
                    out=asq[:], in_=pa[:], func=mybir.ActivationFunctionType.Square
                )
                nc.vector.tensor_tensor(
                    out=P["cext"][0:D, :], in0=asq[:], in1=pb[:],
                    op=mybir.AluOpType.subtract,
                )
                h3t = actp.tile([128, (H3 // 128) * NT], dt.bfloat16, tag="h3t")
                for m in range(H3 // 128):
                    pl = plp.tile([128, NT], dt.float32, tag="pl")
                    for c in range(H2 // 128):
                        nc.tensor.matmul(
                            out=pl[:],
                            lhsT=w3sb[:, c * H3 + m * 128:c * H3 + (m + 1) * 128],
                            rhs=h2t[:, c * NT:(c + 1) * NT],
                            start=(c == 0),
                            stop=(c == H2 // 128 - 1),
                        )
                    nc.scalar.activation(
                        out=h3t[:, m * NT:(m + 1) * NT],
                        in_=pl[:],
                        func=mybir.ActivationFunctionType.Relu,
                    )

                # ---- prev: final row ----
                pfin = pfinp.tile([1, NT], dt.float32)
                nc.tensor.matmul(
                    out=pfin[:],
                    lhsT=wldsb[:],
                    rhs=densesb[:, prev * NT:(prev + 1) * NT],
                    start=True,
                    stop=False,
                )
                for m in range(H3 // 128):
                    nc.tensor.matmul(
                        out=pfin[:],
                        lhsT=woutsb[:, m:m + 1],
                        rhs=h3t[:, m * NT:(m + 1) * NT],
                        start=False,
                        stop=False,
                    )
                nc.tensor.matmul(
                    out=pfin[:],
                    lhsT=coeffsb[0:KFIN, :],
                    rhs=P["cext"][0:KFIN, :],
                    start=False,
                    stop=True,
                )
                row = miscp.tile([1, NT], dt.float32, tag="row")
                nc.vector.tensor_copy(out=row[:], in_=pfin[:])
                nc.sync.dma_start(out=out[prev:prev + 1, :], in_=row[:])
                del H[prev]

    nc.finalize()
    return nc


def _prepare(dense, sparse_idx, bias, emb_tables, lin_tables, Wd, Wld, bld, W1, W2, W3, Wout):
    bf16 = ml_dtypes.bfloat16
    dense = np.asarray(dense, np.float32)
    sparse_idx = np.asarray(sparse_idx)
    table = np.zeros([FV, RSTRIDE], dtype=bf16)
    table[:, 0:D] = np.asarray(emb_tables, np.float32).reshape(FV, D).astype(bf16)
    table[:, D] = np.asarray(lin_tables, np.float32).reshape(FV).astype(bf16)
    wdcat = np.asarray(Wd, np.float32)
    wldv = np.asarray(Wld, np.float32).reshape(ND, 1)
    coeff = np.zeros([128, 1], np.float32)
    coeff[0:D, 0] = 0.5
    coeff[D:D + F, 0] = 1.0
    coeff[96, 0] = float(np.asarray(bias, np.float32).reshape(-1)[0]) + float(
        np.asarray(bld, np.float32).reshape(-1)[0]
    )
    stacki = np.tile(np.eye(D, dtype=bf16), (2, 1))
    ident = np.eye(128, dtype=bf16)
    off = (sparse_idx.astype(np.int64) + (np.arange(F, dtype=np.int64) * V)[None, :]).astype(np.int32)

    shared = {
        "table": table,
        "wdcat": wdcat.astype(np.float32),
        "wld": wldv,
        "w1": np.asarray(W1, np.float32).astype(bf16),
        "w2": np.asarray(W2, np.float32).astype(bf16),
        "w3": np.asarray(W3, np.float32).astype(bf16),
        "wout": np.asarray(Wout, np.float32).astype(bf16),
        "coeff": coeff,
        "stacki": stacki,
        "ident": ident,
    }
    in_maps = []
    for i in range(NCORES):
        sl = slice(i * BS, (i + 1) * BS)
        m = dict(shared)
        m["denseT"] = np.ascontiguousarray(dense[sl].T)
        m["idx"] = np.ascontiguousarray(off[sl])
        in_maps.append(m)
    return in_maps


def kernel(**inputs):
    from concourse.bass_utils import run_bass_kernel_spmd

    in_maps = _prepare(**inputs)
    if "nc" not in _cache:
        _cache["nc"] = _build_nc()
    res = run_bass_kernel_spmd(_cache["nc"], in_maps, list(range(NCORES)))
    outs = [r["out"].reshape(BS, 1).astype(np.float32) for r in res.results]
    return np.concatenate(outs, axis=0)



# revision 13
# speedup vs baseline: 1.8542x; 1.8542x over previous
"""DeepFM Trainium2 kernel — 8-core SPMD, batch-sharded, fp8 MLP.

Strategy: shard the batch (16384 -> 8 x 2048); replicate the (host-packed,
fp8e4m3, x32-scaled) embedding table and MLP weights on every core.  Per core:
  - indirect-DMA gather of 65-fp8-element rows ([64 emb | 1 lin], both
    pre-scaled by 32) from a [F*V, 128]-strided fp8 table, sample-major
  - PE transposes (fp8 x fp8-identity matmuls) to feature-major h^T
    [1728, batch] in fp8
  - fp8 DoubleRow MLP (1728->1024->512->256->1): weights x32 in fp8, each
    ReLU copy rescales by 1/32 (relu is positively homogeneous), so h1/h2
    stay at 32x in fp8 and h3 is exact-scale bf16
  - FM second order via stacked-identity DoubleRow matmuls: A_s=sum_f (32e),
    B_s=sum_f (32e)^2; second = 0.5/1024 * sum_d(A_s^2 - B_s) folded into
    the final 1-row matmul via the coeff vector
  - first order: fp32 matmul of [Wd|Wld] against dense^T; 32x lin-sum rows
    get coeff 1/32; bias/bld folded into the final matmul's contraction rows
"""

import numpy as np
import ml_dtypes

B, F, V, D, ND = 16384, 26, 100000, 64, 13
H1, H2, H3 = 1024, 512, 256
NCORES = 8
BS = B // NCORES            # 2048 samples per core
SUB = 128                   # gather sub-tile (samples)
NT = 512                    # outer batch tile (matmul N)
NOUT = BS // NT             # 4 outer tiles per core
NSUB = NT // SUB            # 4 sub-tiles per outer tile
ROWE = D + 1                # gathered elements per row (64 emb + 1 lin)
RSTRIDE = 128               # fp8 table row stride in elements (128B-aligned)
RUNW = 128                  # per-(s,f) unit width in the gather tile
FV = F * V
HTOT = (F + 1) * D          # 1728
NCH = 14                    # ceil(1728/128); chunk 13 has 64 valid rows
NPAIR = NCH // 2            # DoubleRow contraction pairs
KFIN = 97                   # final misc matmul contraction: 64 C + 26 lin + pad + bias@96
SCALE = 32.0                # fp8 pre-scale on table + W1/W2/W3

_cache = {}


def _build_nc(reps=1):
    import concourse.bass as bass
    import concourse.bacc as bacc
    import concourse.mybir as mybir
    import concourse.tile as tile

    dt = mybir.dt
    DR = mybir.MatmulPerfMode.DoubleRow
    nc = bacc.Bacc()

    denseT = nc.declare_dram_parameter("denseT", [ND, BS], dt.float32, isOutput=False)
    idx = nc.declare_dram_parameter("idx", [BS, F], dt.int32, isOutput=False)
    table = nc.declare_dram_parameter("table", [FV, RSTRIDE], dt.float8e4, isOutput=False)
    wdcat = nc.declare_dram_parameter("wdcat", [ND, D], dt.float32, isOutput=False)
    wld = nc.declare_dram_parameter("wld", [ND, 1], dt.float32, isOutput=False)
    w1 = nc.declare_dram_parameter("w1", [NCH * 128, H1], dt.float8e4, isOutput=False)
    w2 = nc.declare_dram_parameter("w2", [H1, H2], dt.float8e4, isOutput=False)
    w3 = nc.declare_dram_parameter("w3", [H2, H3], dt.float8e4, isOutput=False)
    wout = nc.declare_dram_parameter("wout", [H3, 1], dt.bfloat16, isOutput=False)
    coeff = nc.declare_dram_parameter("coeff", [128, 1], dt.float32, isOutput=False)
    stacki = nc.declare_dram_parameter("stacki", [128, 2 * D], dt.float8e4, isOutput=False)
    ident = nc.declare_dram_parameter("ident", [128, 128], dt.float8e4, isOutput=False)
    out = nc.declare_dram_parameter("out", [NOUT, NT], dt.float32, isOutput=True)

    with tile.TileContext(nc) as tc:
        with (
            tc.tile_pool(name="const", bufs=1) as constp,
            tc.tile_pool(name="g", bufs=2) as gp,
            tc.tile_pool(name="ht", bufs=2) as htp,
            tc.tile_pool(name="act", bufs=2) as actp,
            tc.tile_pool(name="sq", bufs=3) as sqp,
            tc.tile_pool(name="misc", bufs=2) as miscp,
            tc.tile_pool(name="pdcat", bufs=1, space="PSUM") as pdcatp,
            tc.tile_pool(name="ptr", bufs=2, space="PSUM") as ptrp,
            tc.tile_pool(name="pab", bufs=1, space="PSUM") as pabp,
            tc.tile_pool(name="pfin", bufs=1, space="PSUM") as pfinp,
            tc.tile_pool(name="pl", bufs=2, space="PSUM") as plp,
        ):
            # ---- constants / weights, loaded once ----
            densesb = constp.tile([ND, BS], dt.float32)
            nc.sync.dma_start(out=densesb[:], in_=denseT[:])
            idxsb = constp.tile([SUB, (BS // SUB) * F], dt.int32)
            nc.sync.dma_start(
                out=idxsb[:].rearrange("p (st f) -> p st f", f=F),
                in_=idx[:].rearrange("(st p) f -> p st f", p=SUB),
            )
            wdcatsb = constp.tile([ND, D], dt.float32)
            nc.sync.dma_start(out=wdcatsb[:], in_=wdcat[:])
            wldsb = constp.tile([ND, 1], dt.float32)
            nc.sync.dma_start(out=wldsb[:], in_=wld[:])
            w1sb = constp.tile([128, NCH * H1], dt.float8e4)
            for c in range(NCH):
                nc.sync.dma_start(
                    out=w1sb[:, c * H1:(c + 1) * H1],
                    in_=w1[c * 128:(c + 1) * 128, :],
                )
            w2sb = constp.tile([128, (H1 // 128) * H2], dt.float8e4)
            for c in range(H1 // 128):
                nc.sync.dma_start(
                    out=w2sb[:, c * H2:(c + 1) * H2],
                    in_=w2[c * 128:(c + 1) * 128, :],
                )
            w3sb = constp.tile([128, (H2 // 128) * H3], dt.float8e4)
            for c in range(H2 // 128):
                nc.sync.dma_start(
                    out=w3sb[:, c * H3:(c + 1) * H3],
                    in_=w3[c * 128:(c + 1) * 128, :],
                )
            woutsb = constp.tile([128, H3 // 128], dt.bfloat16)
            nc.sync.dma_start(
                out=woutsb[:], in_=wout[:].rearrange("(c p) one -> p (c one)", p=128)
            )
            coeffsb = constp.tile([128, 1], dt.float32)
            nc.sync.dma_start(out=coeffsb[:], in_=coeff[:])
            stackisb = constp.tile([128, 2 * D], dt.float8e4)
            nc.sync.dma_start(out=stackisb[:], in_=stacki[:])
            identsb = constp.tile([128, 128], dt.float8e4)
            nc.sync.dma_start(out=identsb[:], in_=ident[:])

            w1r = w1sb[:].rearrange("p (c h) -> p c h", h=H1)
            w2r = w2sb[:].rearrange("p (c h) -> p c h", h=H2)
            w3r = w3sb[:].rearrange("p (c h) -> p c h", h=H3)
            stk = stackisb[:].rearrange("p (j m) -> p j m", m=D)

            # ================= software-pipelined tile loop =================
            # Stage k runs the "front" of tile cur (gathers, dense matmul,
            # PE transposes to feature-major) interleaved with the "compute"
            # of tile prev (FM sums + MLP + final row).
            tiles = [t for _ in range(reps) for t in range(NOUT)]
            steps = [(tiles[k], tiles[k - 1] if k else None) for k in range(len(tiles))]
            steps.append((None, tiles[-1]))
            H = {}   # live handles for the in-flight tile

            def chunk_feats(c):
                if c == 0:
                    return D, D, [(D, 0)]
                if c == NCH - 1:
                    return 0, D, [(0, F - 1)]
                return 0, 128, [(0, 2 * c - 1), (D, 2 * c)]

            for cur, prev in steps:
                P = H.get(prev)
                if cur is not None:
                    # one gather per (feature, sub-tile): HW indirect DMA uses
                    # ONE index per partition and copies the out free-run
                    # contiguously, so each call fetches 65 elems ([64 emb |
                    # 1 lin] are adjacent in the table row) for 128 samples.
                    g = gp.tile([SUB, NSUB * F * RUNW], dt.float8e4, tag="g")
                    for f in range(F):
                        for s in range(NSUB):
                            st = cur * NSUB + s
                            base = (s * F + f) * RUNW
                            nc.gpsimd.indirect_dma_start(
                                out=g[:, base:base + ROWE],
                                out_offset=None,
                                in_=table[:],
                                in_offset=bass.IndirectOffsetOnAxis(
                                    ap=idxsb[:, st * F + f:st * F + f + 1], axis=0
                                ),
                            )
                    pdcat = pdcatp.tile([D, NT], dt.float32)
                    nc.tensor.matmul(
                        out=pdcat[:],
                        lhsT=wdcatsb[:],
                        rhs=densesb[:, cur * NT:(cur + 1) * NT],
                        start=True,
                        stop=True,
                    )
                    ht = htp.tile([128, NCH * NT], dt.float8e4, tag="ht")
                    nc.scalar.activation(
                        out=ht[0:D, 0:NT],
                        in_=pdcat[0:D, :],
                        func=mybir.ActivationFunctionType.Copy,
                        scale=SCALE,
                    )
                    # zero the dead rows of the last (64-row) chunk so the
                    # DoubleRow pair (12,13) contracts over clean zeros
                    nc.vector.memset(ht[D:128, (NCH - 1) * NT:NCH * NT], 0.0)
                    cext = miscp.tile([128, NT], dt.float32, tag="cext")
                    nc.vector.memset(cext[D:128, :], 0.0)
                    nc.vector.memset(cext[96:97, :], 1.0)
                    C = {"g": g, "ht": ht, "cext": cext}
                    H[cur] = C
                if P is not None:
                    pa = pabp.tile([D, NT], dt.float32, tag="pa")
                    pb = pabp.tile([D, NT], dt.float32, tag="pb")
                    h1t = actp.tile([128, (H1 // 128) * NT], dt.float8e4, tag="h1t")
                    htr = P["ht"][:].rearrange("p (c n) -> p c n", n=NT)

                # interleaved: per chunk c, prev's sq/A/B + cur's transposes,
                # plus one L1 m-tile for the first 8 chunks
                for c in range(NCH):
                    if P is not None:
                        htc = P["ht"][0:128, c * NT:(c + 1) * NT]
                        if c % 2 == 0:
                            sq2 = sqp.tile([128, 2 * NT], dt.float8e4, tag="sq")
                        nc.vector.tensor_tensor(
                            out=sq2[:, (c % 2) * NT:(c % 2 + 1) * NT],
                            in0=htc, in1=htc,
                            op=mybir.AluOpType.mult,
                        )
                        if c % 2 == 1:
                            cp = c // 2
                            sq2r = sq2[:].rearrange("p (j n) -> p j n", n=NT)
                            nc.tensor.matmul(
                                out=pa[:], lhsT=stk[:, 0:2, :],
                                rhs=htr[:, c - 1:c + 1, :],
                                start=(cp == 0), stop=(cp == NPAIR - 1),
                                perf_mode=DR,
                            )
                            nc.tensor.matmul(
                                out=pb[:], lhsT=stk[:, 0:2, :],
                                rhs=sq2r[:, 0:2, :],
                                start=(cp == 0), stop=(cp == NPAIR - 1),
                                perf_mode=DR,
                            )
                    if cur is not None:
                        # transpose via REGULAR matmul G.T @ I (fp8)
                        plo, kcc, feats = chunk_feats(c)
                        ptr = ptrp.tile([128, NT], dt.float32, tag="ptr")
                        gv = C["g"][:].rearrange("p (i e) -> p i e", e=RUNW)
                        for s in range(NSUB):
                            for ro, fi in feats:
                                nc.tensor.matmul(
                                    out=ptr[plo + ro - feats[0][0]:
                                            plo + ro - feats[0][0] + D,
                                            s * SUB:(s + 1) * SUB],
                                    lhsT=gv[:, s * F + fi:s * F + fi + 1, 0:D],
                                    rhs=identsb[:],
                                    start=True,
                                    stop=True,
                                )
                        nc.vector.tensor_copy(
                            out=C["ht"][plo:plo + kcc, c * NT:(c + 1) * NT],
                            in_=ptr[plo:plo + kcc, :],
                        )
                    if P is not None and c < H1 // 128:
                        m = c
                        pl = plp.tile([128, NT], dt.float32, tag="pl")
                        for cp in range(NPAIR):
                            nc.tensor.matmul(
                                out=pl[:],
                                lhsT=w1r[:, 2 * cp:2 * cp + 2, m * 128:(m + 1) * 128],
                                rhs=htr[:, 2 * cp:2 * cp + 2, :],
                                start=(cp == 0),
                                stop=(cp == NPAIR - 1),
                                perf_mode=DR,
                            )
                        nc.scalar.activation(
                            out=h1t[:, m * NT:(m + 1) * NT],
                            in_=pl[:],
                            func=mybir.ActivationFunctionType.Relu,
                            scale=1.0 / SCALE,
                        )

                # cur: lin-row transposes (all gathers are done by now)
                if cur is not None:
                    pltr = ptrp.tile([128, NT], dt.float32, tag="ptr")
                    gv = C["g"][:].rearrange("p (i e) -> p i e", e=RUNW)
                    for s in range(NSUB):
                        nc.tensor.matmul(
                            out=pltr[D:D + F, s * SUB:(s + 1) * SUB],
                            lhsT=gv[:, s * F:(s + 1) * F, D:D + 1],
                            rhs=identsb[:],
                            start=True,
                            stop=True,
                        )
                    nc.vector.tensor_copy(
                        out=C["cext"][D:D + F, :], in_=pltr[D:D + F, :]
                    )

                if P is None:
                    continue

                # ---- prev: layers 2/3 ----
                h1tr = h1t[:].rearrange("p (c n) -> p c n", n=NT)
                h2t = actp.tile([128, (H2 // 128) * NT], dt.float8e4, tag="h2t")
                for m in range(H2 // 128):
                    pl = plp.tile([128, NT], dt.float32, tag="pl")
                    for cp in range(H1 // 256):
                        nc.tensor.matmul(
                            out=pl[:],
                            lhsT=w2r[:, 2 * cp:2 * cp + 2, m * 128:(m + 1) * 128],
                            rhs=h1tr[:, 2 * cp:2 * cp + 2, :],
                            start=(cp == 0),
                            stop=(cp == H1 // 256 - 1),
                            perf_mode=DR,
                        )
                    nc.scalar.activation(
                        out=h2t[:, m * NT:(m + 1) * NT],
                        in_=pl[:],
                        func=mybir.ActivationFunctionType.Relu,
                        scale=1.0 / SCALE,
                    )
                # FM second-order combine, overlaps L3 on PE
                asq = miscp.tile([D, NT], dt.float32, tag="asq")
                nc.scalar.activation(
                    out=asq[:], in_=pa[:], func=mybir.ActivationFunctionType.Square
                )
                nc.vector.tensor_tensor(
                    out=P["cext"][0:D, :], in0=asq[:], in1=pb[:],
                    op=mybir.AluOpType.subtract,
                )
                h2tr = h2t[:].rearrange("p (c n) -> p c n", n=NT)
                h3t = actp.tile([128, (H3 // 128) * NT], dt.bfloat16, tag="h3t")
                for m in range(H3 // 128):
                    pl = plp.tile([128, NT], dt.float32, tag="pl")
                    for cp in range(H2 // 256):
                        nc.tensor.matmul(
                            out=pl[:],
                            lhsT=w3r[:, 2 * cp:2 * cp + 2, m * 128:(m + 1) * 128],
                            rhs=h2tr[:, 2 * cp:2 * cp + 2, :],
                            start=(cp == 0),
                            stop=(cp == H2 // 256 - 1),
                            perf_mode=DR,
                        )
                    nc.scalar.activation(
                        out=h3t[:, m * NT:(m + 1) * NT],
                        in_=pl[:],
                        func=mybir.ActivationFunctionType.Relu,
                        scale=1.0 / (SCALE * SCALE),
                    )

                # ---- prev: final row ----
                pfin = pfinp.tile([1, NT], dt.float32)
                nc.tensor.matmul(
                    out=pfin[:],
                    lhsT=wldsb[:],
                    rhs=densesb[:, prev * NT:(prev + 1) * NT],
                    start=True,
                    stop=False,
                )
                for m in range(H3 // 128):
                    nc.tensor.matmul(
                        out=pfin[:],
                        lhsT=woutsb[:, m:m + 1],
                        rhs=h3t[:, m * NT:(m + 1) * NT],
                        start=False,
                        stop=False,
                    )
                nc.tensor.matmul(
                    out=pfin[:],
                    lhsT=coeffsb[0:KFIN, :],
                    rhs=P["cext"][0:KFIN, :],
                    start=False,
                    stop=True,
                )
                row = miscp.tile([1, NT], dt.float32, tag="row")
                nc.vector.tensor_copy(out=row[:], in_=pfin[:])
                nc.sync.dma_start(out=out[prev:prev + 1, :], in_=row[:])
                del H[prev]

    nc.finalize()
    return nc


def _prepare(dense, sparse_idx, bias, emb_tables, lin_tables, Wd, Wld, bld, W1, W2, W3, Wout):
    bf16 = ml_dtypes.bfloat16
    fp8 = ml_dtypes.float8_e4m3
    dense = np.asarray(dense, np.float32)
    sparse_idx = np.asarray(sparse_idx)
    table = np.zeros([FV, RSTRIDE], dtype=fp8)
    table[:, 0:D] = (np.asarray(emb_tables, np.float32).reshape(FV, D) * SCALE).astype(fp8)
    table[:, D] = (np.asarray(lin_tables, np.float32).reshape(FV) * SCALE).astype(fp8)
    wdcat = np.asarray(Wd, np.float32)
    wldv = np.asarray(Wld, np.float32).reshape(ND, 1)
    coeff = np.zeros([128, 1], np.float32)
    coeff[0:D, 0] = 0.5 / (SCALE * SCALE)
    coeff[D:D + F, 0] = 1.0 / SCALE
    coeff[96, 0] = float(np.asarray(bias, np.float32).reshape(-1)[0]) + float(
        np.asarray(bld, np.float32).reshape(-1)[0]
    )
    st64 = np.tile(np.eye(D, dtype=np.float32), (2, 1))
    stacki = np.concatenate([st64, st64], axis=1).astype(fp8)
    ident = np.eye(128, dtype=fp8)
    w1p = np.zeros([NCH * 128, H1], dtype=fp8)
    w1p[0:HTOT] = (np.asarray(W1, np.float32) * SCALE).astype(fp8)
    off = (sparse_idx.astype(np.int64) + (np.arange(F, dtype=np.int64) * V)[None, :]).astype(np.int32)

    shared = {
        "table": table,
        "wdcat": wdcat.astype(np.float32),
        "wld": wldv,
        "w1": w1p,
        "w2": (np.asarray(W2, np.float32) * SCALE).astype(fp8),
        "w3": (np.asarray(W3, np.float32) * SCALE).astype(fp8),
        "wout": np.asarray(Wout, np.float32).astype(bf16),
        "coeff": coeff,
        "stacki": stacki,
        "ident": ident,
    }
    in_maps = []
    for i in range(NCORES):
        sl = slice(i * BS, (i + 1) * BS)
        m = dict(shared)
        m["denseT"] = np.ascontiguousarray(dense[sl].T)
        m["idx"] = np.ascontiguousarray(off[sl])
        in_maps.append(m)
    return in_maps


def kernel(**inputs):
    from concourse.bass_utils import run_bass_kernel_spmd

    in_maps = _prepare(**inputs)
    if "nc" not in _cache:
        _cache["nc"] = _build_nc()
    res = run_bass_kernel_spmd(_cache["nc"], in_maps, list(range(NCORES)))
    outs = [r["out"].reshape(BS, 1).astype(np.float32) for r in res.results]
    return np.concatenate(outs, axis=0)


# revision 17
# speedup vs baseline: 1.8651x; 1.0059x over previous
"""DeepFM Trainium2 kernel — 8-core SPMD, batch-sharded.

Strategy: shard the batch (16384 -> 8 x 2048); replicate the (host-packed,
bf16) embedding table and MLP weights on every core.  Per core:
  - indirect-DMA gather of 65-bf16-element rows ([64 emb | 1 lin]) from a
    combined [F*V, 96]-strided table, sample-major in SBUF
  - PE transposes to feature-major h^T [1728, batch]
  - bf16 MLP (1728->1024->512->256->1) feature-major, fp32 PSUM accumulate
  - FM second order via stacked-identity matmuls: A=sum_f e, B=sum_f e^2,
    second = 0.5*sum_d(A^2 - B) folded into the final 1-row matmul
  - first order: fp32 matmul of [Wd|Wld] against dense^T; lin-sum and
    bias/bld folded into the final matmul's contraction rows
"""

import numpy as np
import ml_dtypes

B, F, V, D, ND = 16384, 26, 100000, 64, 13
H1, H2, H3 = 1024, 512, 256
NCORES = 8
BS = B // NCORES            # 2048 samples per core
SUB = 128                   # gather sub-tile (samples)
NT = 512                    # outer batch tile (matmul N)
NOUT = BS // NT             # 4 outer tiles per core
NSUB = NT // SUB            # 4 sub-tiles per outer tile
ROWE = D + 1                # gathered elements per row (64 emb + 1 lin)
RSTRIDE = 96                # table row stride in elements (192B, 64B-aligned)
RUNW = 96                   # per-(s,f) unit width in the gather tile (64B-aligned)
FV = F * V
HTOT = (F + 1) * D          # 1728
NCH = 14                    # ceil(1728/128); chunk 13 has 64 rows
KFIN = 97                   # final misc matmul contraction: 64 C + 26 lin + pad + bias@96

_cache = {}


def _build_nc(reps=1):
    import concourse.bass as bass
    import concourse.bacc as bacc
    import concourse.mybir as mybir
    import concourse.tile as tile

    dt = mybir.dt
    nc = bacc.Bacc()

    denseT = nc.declare_dram_parameter("denseT", [ND, BS], dt.float32, isOutput=False)
    idx = nc.declare_dram_parameter("idx", [BS, F], dt.int32, isOutput=False)
    table = nc.declare_dram_parameter("table", [FV, RSTRIDE], dt.bfloat16, isOutput=False)
    wdcat = nc.declare_dram_parameter("wdcat", [ND, D], dt.float32, isOutput=False)
    wld = nc.declare_dram_parameter("wld", [ND, 1], dt.float32, isOutput=False)
    w1 = nc.declare_dram_parameter("w1", [HTOT, H1], dt.bfloat16, isOutput=False)
    w2 = nc.declare_dram_parameter("w2", [H1, H2], dt.bfloat16, isOutput=False)
    w3 = nc.declare_dram_parameter("w3", [H2, H3], dt.bfloat16, isOutput=False)
    wout = nc.declare_dram_parameter("wout", [H3, 1], dt.bfloat16, isOutput=False)
    coeff = nc.declare_dram_parameter("coeff", [128, 1], dt.float32, isOutput=False)
    stacki = nc.declare_dram_parameter("stacki", [128, D], dt.bfloat16, isOutput=False)
    ident = nc.declare_dram_parameter("ident", [128, 128], dt.bfloat16, isOutput=False)
    out = nc.declare_dram_parameter("out", [NOUT, NT], dt.float32, isOutput=True)

    with tile.TileContext(nc) as tc:
        with (
            tc.tile_pool(name="const", bufs=1) as constp,
            tc.tile_pool(name="g", bufs=2) as gp,
            tc.tile_pool(name="ht", bufs=2) as htp,
            tc.tile_pool(name="act", bufs=2) as actp,
            tc.tile_pool(name="sq", bufs=3) as sqp,
            tc.tile_pool(name="misc", bufs=2) as miscp,
            tc.tile_pool(name="pdcat", bufs=1, space="PSUM") as pdcatp,
            tc.tile_pool(name="ptr", bufs=2, space="PSUM") as ptrp,
            tc.tile_pool(name="pab", bufs=1, space="PSUM") as pabp,
            tc.tile_pool(name="pfin", bufs=1, space="PSUM") as pfinp,
            tc.tile_pool(name="pl", bufs=2, space="PSUM") as plp,
        ):
            # ---- constants / weights, loaded once ----
            densesb = constp.tile([ND, BS], dt.float32)
            nc.sync.dma_start(out=densesb[:], in_=denseT[:])
            idxsb = constp.tile([SUB, (BS // SUB) * F], dt.int32)
            nc.sync.dma_start(
                out=idxsb[:].rearrange("p (st f) -> p st f", f=F),
                in_=idx[:].rearrange("(st p) f -> p st f", p=SUB),
            )
            wdcatsb = constp.tile([ND, D], dt.float32)
            nc.sync.dma_start(out=wdcatsb[:], in_=wdcat[:])
            wldsb = constp.tile([ND, 1], dt.float32)
            nc.sync.dma_start(out=wldsb[:], in_=wld[:])
            w1sb = constp.tile([128, NCH * H1], dt.bfloat16)
            for c in range(NCH):
                kc = min(128, HTOT - c * 128)
                nc.sync.dma_start(
                    out=w1sb[:kc, c * H1:(c + 1) * H1],
                    in_=w1[c * 128:c * 128 + kc, :],
                )
            w2sb = constp.tile([128, (H1 // 128) * H2], dt.bfloat16)
            for c in range(H1 // 128):
                nc.sync.dma_start(
                    out=w2sb[:, c * H2:(c + 1) * H2],
                    in_=w2[c * 128:(c + 1) * 128, :],
                )
            w3sb = constp.tile([128, (H2 // 128) * H3], dt.bfloat16)
            for c in range(H2 // 128):
                nc.sync.dma_start(
                    out=w3sb[:, c * H3:(c + 1) * H3],
                    in_=w3[c * 128:(c + 1) * 128, :],
                )
            woutsb = constp.tile([128, H3 // 128], dt.bfloat16)
            nc.sync.dma_start(
                out=woutsb[:], in_=wout[:].rearrange("(c p) one -> p (c one)", p=128)
            )
            coeffsb = constp.tile([128, 1], dt.float32)
            nc.sync.dma_start(out=coeffsb[:], in_=coeff[:])
            stackisb = constp.tile([128, D], dt.bfloat16)
            nc.sync.dma_start(out=stackisb[:], in_=stacki[:])
            identsb = constp.tile([128, 128], dt.bfloat16)
            nc.sync.dma_start(out=identsb[:], in_=ident[:])

            # ================= software-pipelined tile loop =================
            # Stage k runs the "front" of tile cur (gathers, dense matmul,
            # PE transposes to feature-major) interleaved with the "compute"
            # of tile prev (FM sums + MLP + final row), so the PE never has
            # long idle gaps (keeps HAM at 8/8) and DVE copies hide under
            # matmul phases.
            tiles = [t for _ in range(reps) for t in range(NOUT)]
            steps = [(tiles[k], tiles[k - 1] if k else None) for k in range(len(tiles))]
            steps.append((None, tiles[-1]))
            H = {}   # live handles for the in-flight tile

            def chunk_feats(c):
                if c == 0:
                    return D, D, [(D, 0)]
                if c == NCH - 1:
                    return 0, D, [(0, F - 1)]
                return 0, 128, [(0, 2 * c - 1), (D, 2 * c)]

            for cur, prev in steps:
                P = H.get(prev)
                if cur is not None:
                    # one gather per (feature, sub-tile): HW indirect DMA uses
                    # ONE index per partition and copies the out free-run
                    # contiguously, so each call fetches 65 elems ([64 emb |
                    # 1 lin] are adjacent in the table row) for 128 samples.
                    g = gp.tile([SUB, NSUB * F * RUNW], dt.bfloat16, tag="g")
                    for f in range(F):
                        for s in range(NSUB):
                            st = cur * NSUB + s
                            base = (s * F + f) * RUNW
                            nc.gpsimd.indirect_dma_start(
                                out=g[:, base:base + ROWE],
                                out_offset=None,
                                in_=table[:],
                                in_offset=bass.IndirectOffsetOnAxis(
                                    ap=idxsb[:, st * F + f:st * F + f + 1], axis=0
                                ),
                            )
                    pdcat = pdcatp.tile([D, NT], dt.float32)
                    nc.tensor.matmul(
                        out=pdcat[:],
                        lhsT=wdcatsb[:],
                        rhs=densesb[:, cur * NT:(cur + 1) * NT],
                        start=True,
                        stop=True,
                    )
                    ht = htp.tile([128, NCH * NT], dt.bfloat16, tag="ht")
                    nc.scalar.activation(
                        out=ht[0:D, 0:NT],
                        in_=pdcat[0:D, :],
                        func=mybir.ActivationFunctionType.Copy,
                    )
                    cext = miscp.tile([128, NT], dt.float32, tag="cext")
                    nc.vector.memset(cext[D:128, :], 0.0)
                    nc.vector.memset(cext[96:97, :], 1.0)
                    C = {"g": g, "ht": ht, "cext": cext}
                    H[cur] = C
                if P is not None:
                    pa = pabp.tile([D, NT], dt.float32, tag="pa")
                    pb = pabp.tile([D, NT], dt.float32, tag="pb")
                    h1t = actp.tile([128, (H1 // 128) * NT], dt.bfloat16, tag="h1t")

                # interleaved: per chunk c, prev's sq/A/B + cur's transposes,
                # plus one L1 m-tile for the first 8 chunks
                for c in range(NCH):
                    kc = min(128, HTOT - c * 128)
                    if P is not None:
                        htc = P["ht"][0:kc, c * NT:(c + 1) * NT]
                        sq = sqp.tile([128, NT], dt.bfloat16, tag="sq")
                        nc.vector.tensor_tensor(
                            out=sq[0:kc, :], in0=htc, in1=htc,
                            op=mybir.AluOpType.mult,
                        )
                        nc.tensor.matmul(
                            out=pa[:], lhsT=stackisb[0:kc, :], rhs=htc,
                            start=(c == 0), stop=(c == NCH - 1),
                        )
                        nc.tensor.matmul(
                            out=pb[:], lhsT=stackisb[0:kc, :], rhs=sq[0:kc, :],
                            start=(c == 0), stop=(c == NCH - 1),
                        )
                    if cur is not None:
                        # transpose via REGULAR matmul G.T @ I — unlike
                        # transpose-mode this counts as PE-busy for HAM
                        plo, kcc, feats = chunk_feats(c)
                        f0 = feats[0][1]
                        ptr = ptrp.tile([128, NT], dt.float32, tag="ptr")
                        gv = C["g"][:].rearrange("p (i e) -> p i e", e=RUNW)
                        for s in range(NSUB):
                            for ro, fi in feats:
                                nc.tensor.matmul(
                                    out=ptr[plo + ro - feats[0][0]:
                                            plo + ro - feats[0][0] + D,
                                            s * SUB:(s + 1) * SUB],
                                    lhsT=gv[:, s * F + fi:s * F + fi + 1, 0:D],
                                    rhs=identsb[:],
                                    start=True,
                                    stop=True,
                                )
                        nc.vector.tensor_copy(
                            out=C["ht"][plo:plo + kcc, c * NT:(c + 1) * NT],
                            in_=ptr[plo:plo + kcc, :],
                        )
                    if P is not None and c < H1 // 128:
                        m = c
                        pl = plp.tile([128, NT], dt.float32, tag="pl")
                        for cc in range(NCH):
                            kcc2 = min(128, HTOT - cc * 128)
                            nc.tensor.matmul(
                                out=pl[:],
                                lhsT=w1sb[0:kcc2, cc * H1 + m * 128:cc * H1 + (m + 1) * 128],
                                rhs=P["ht"][0:kcc2, cc * NT:(cc + 1) * NT],
                                start=(cc == 0),
                                stop=(cc == NCH - 1),
                            )
                        nc.scalar.activation(
                            out=h1t[:, m * NT:(m + 1) * NT],
                            in_=pl[:],
                            func=mybir.ActivationFunctionType.Relu,
                        )

                # cur: lin-row transposes (all gathers are done by now)
                if cur is not None:
                    pltr = ptrp.tile([128, NT], dt.float32, tag="ptr")
                    gv = C["g"][:].rearrange("p (i e) -> p i e", e=RUNW)
                    for s in range(NSUB):
                        nc.tensor.matmul(
                            out=pltr[D:D + F, s * SUB:(s + 1) * SUB],
                            lhsT=gv[:, s * F:(s + 1) * F, D:D + 1],
                            rhs=identsb[:],
                            start=True,
                            stop=True,
                        )
                    nc.vector.tensor_copy(
                        out=C["cext"][D:D + F, :], in_=pltr[D:D + F, :]
                    )

                if P is None:
                    continue

                # ---- prev: layers 2/3 ----
                h2t = actp.tile([128, (H2 // 128) * NT], dt.bfloat16, tag="h2t")
                for m in range(H2 // 128):
                    pl = plp.tile([128, NT], dt.float32, tag="pl")
                    for c in range(H1 // 128):
                        nc.tensor.matmul(
                            out=pl[:],
                            lhsT=w2sb[:, c * H2 + m * 128:c * H2 + (m + 1) * 128],
                            rhs=h1t[:, c * NT:(c + 1) * NT],
                            start=(c == 0),
                            stop=(c == H1 // 128 - 1),
                        )
                    nc.scalar.activation(
                        out=h2t[:, m * NT:(m + 1) * NT],
                        in_=pl[:],
                        func=mybir.ActivationFunctionType.Relu,
                    )
                # FM second-order combine, overlaps L3 on PE
                asq = miscp.tile([D, NT], dt.float32, tag="asq")
                nc.scalar.activation(
                    out=asq[:], in_=pa[:], func=mybir.ActivationFunctionType.Square
                )
                nc.vector.tensor_tensor(
                    out=P["cext"][0:D, :], in0=asq[:], in1=pb[:],
                    op=mybir.AluOpType.subtract,
                )
                h3t = actp.tile([128, (H3 // 128) * NT], dt.bfloat16, tag="h3t")
                for m in range(H3 // 128):
                    pl = plp.tile([128, NT], dt.float32, tag="pl")
                    for c in range(H2 // 128):
                        nc.tensor.matmul(
                            out=pl[:],
                            lhsT=w3sb[:, c * H3 + m * 128:c * H3 + (m + 1) * 128],
                            rhs=h2t[:, c * NT:(c + 1) * NT],
                            start=(c == 0),
                            stop=(c == H2 // 128 - 1),
                        )
                    nc.scalar.activation(
                        out=h3t[:, m * NT:(m + 1) * NT],
                        in_=pl[:],
                        func=mybir.ActivationFunctionType.Relu,
                    )

                # ---- prev: final row ----
                pfin = pfinp.tile([1, NT], dt.float32)
                nc.tensor.matmul(
                    out=pfin[:],
                    lhsT=wldsb[:],
                    rhs=densesb[:, prev * NT:(prev + 1) * NT],
                    start=True,
                    stop=False,
                )
                for m in range(H3 // 128):
                    nc.tensor.matmul(
                        out=pfin[:],
                        lhsT=woutsb[:, m:m + 1],
                        rhs=h3t[:, m * NT:(m + 1) * NT],
                        start=False,
                        stop=False,
                    )
                nc.tensor.matmul(
                    out=pfin[:],
                    lhsT=coeffsb[0:KFIN, :],
                    rhs=P["cext"][0:KFIN, :],
                    start=False,
                    stop=True,
                )
                row = miscp.tile([1, NT], dt.float32, tag="row")
                nc.vector.tensor_copy(out=row[:], in_=pfin[:])
                nc.sync.dma_start(out=out[prev:prev + 1, :], in_=row[:])
                del H[prev]

    nc.finalize()
    return nc


def _prepare(dense, sparse_idx, bias, emb_tables, lin_tables, Wd, Wld, bld, W1, W2, W3, Wout):
    bf16 = ml_dtypes.bfloat16
    dense = np.asarray(dense, np.float32)
    sparse_idx = np.asarray(sparse_idx)
    table = np.zeros([FV, RSTRIDE], dtype=bf16)
    table[:, 0:D] = np.asarray(emb_tables, np.float32).reshape(FV, D).astype(bf16)
    table[:, D] = np.asarray(lin_tables, np.float32).reshape(FV).astype(bf16)
    wdcat = np.asarray(Wd, np.float32)
    wldv = np.asarray(Wld, np.float32).reshape(ND, 1)
    coeff = np.zeros([128, 1], np.float32)
    coeff[0:D, 0] = 0.5
    coeff[D:D + F, 0] = 1.0
    coeff[96, 0] = float(np.asarray(bias, np.float32).reshape(-1)[0]) + float(
        np.asarray(bld, np.float32).reshape(-1)[0]
    )
    stacki = np.tile(np.eye(D, dtype=bf16), (2, 1))
    ident = np.eye(128, dtype=bf16)
    off = (sparse_idx.astype(np.int64) + (np.arange(F, dtype=np.int64) * V)[None, :]).astype(np.int32)

    shared = {
        "table": table,
        "wdcat": wdcat.astype(np.float32),
        "wld": wldv,
        "w1": np.asarray(W1, np.float32).astype(bf16),
        "w2": np.asarray(W2, np.float32).astype(bf16),
        "w3": np.asarray(W3, np.float32).astype(bf16),
        "wout": np.asarray(Wout, np.float32).astype(bf16),
        "coeff": coeff,
        "stacki": stacki,
        "ident": ident,
    }
    in_maps = []
    for i in range(NCORES):
        sl = slice(i * BS, (i + 1) * BS)
        m = dict(shared)
        m["denseT"] = np.ascontiguousarray(dense[sl].T)
        m["idx"] = np.ascontiguousarray(off[sl])
        in_maps.append(m)
    return in_maps


def kernel(**inputs):
    from concourse.bass_utils import run_bass_kernel_spmd

    in_maps = _prepare(**inputs)
    if "nc" not in _cache:
        _cache["nc"] = _build_nc()
    res = run_bass_kernel_spmd(_cache["nc"], in_maps, list(range(NCORES)))
    outs = [r["out"].reshape(BS, 1).astype(np.float32) for r in res.results]
    return np.concatenate(outs, axis=0)

